# revision 83
# baseline (speedup 1.0000x reference)
"""Trainium2 Bass kernel for LlamaRALAAttention (B=2, S=4096, HID=2048, NH=16, NKV=4, HD=128).

Sharding: 8 cores = DP(batch=2) x TP(kv-head groups=4). Core c handles batch c//4,
kv group c%4 (4 q heads + 1 kv head). Softmax/mean over S stay core-local.
o_proj partials are written bf16 and summed on host (the only cross-core reduction).

Pipeline (per core, "everything transposed" layout):
  xT [HID,S] host-pretransposed, bf16. Projections stream xT chunks as moving operand.
  q path in [d,s] layout: q^T = Wq_h^T @ xT, RoPE via R-matmul + cos/sin mults,
    kappa=exp(min(x,0))+max(x,0) -> QkT (bf16, resident); Qg partial folded into the
    kappa op via accum_out.
  k/v path in [s,d] layout: lhsT=xT tile (stationary), rhs=[Wk|Wv]; RoPE on free dim;
    kappa -> Kk_sd, v_sd (bf16, resident). KkT via PE transpose.
  All DVE-fed PE ops (KkT transposes, q-rope matmuls) go through a pending FIFO and are
  issued one matmul-group late, so the in-order PE queue never waits on a DVE chain.
  Logits: per-s-tile matvecs batched over the 4 heads into one PSUM bank; softmax
  (exact global max) batched over heads and interleaved into the first phi chunk;
  outer = (alpha*Kk)^T @ v with alpha applied via broadcast muls; result^T: lhsT=outer,
  rhs=QkT; ctx^T = phiT * result^T; o_proj one chunk behind phi/result so the ctx DVE
  muls never stall PE.
"""

import sys

sys.path.insert(0, "/opt/trn_rl_repo")

import numpy as np
import ml_dtypes

import concourse.bass as bass
import concourse.mybir as mybir
import concourse.tile as tile
from concourse import bacc
from concourse.bass_utils import run_bass_kernel_spmd
from concourse.masks import make_identity

P = 128
S = 4096
HID = 2048
HD = 128
NHL = 4            # q heads per core
KO = HID // P      # 16 contraction subtiles
CS = 512           # token chunk size
NCH = S // CS      # 8 chunks
NST = S // P       # 32 s-tiles
HSTEP = NST // 2
ROPE_THETA = 10000.0

F32 = mybir.dt.float32
BF16 = mybir.dt.bfloat16
BF = ml_dtypes.bfloat16

_CACHE = {}


def _build():
    nc = bacc.Bacc("TRN2", target_bir_lowering=False, debug=False, num_devices=8)

    xT = nc.dram_tensor("xT", [HID, S], BF16, kind="ExternalInput").ap()
    csT2 = nc.dram_tensor("csT2", [P, 2, S], F32, kind="ExternalInput").ap()
    css_sd = nc.dram_tensor("css_sd", [S, HD], F32, kind="ExternalInput").ap()
    Wq = nc.dram_tensor("Wq", [HID, NHL * HD], BF16, kind="ExternalInput").ap()
    Wkv = nc.dram_tensor("Wkv", [HID, 2 * HD], BF16, kind="ExternalInput").ap()
    Wphi = nc.dram_tensor("Wphi", [HID, NHL * HD], BF16, kind="ExternalInput").ap()
    Wo = nc.dram_tensor("Wo", [NHL * HD, HID], BF16, kind="ExternalInput").ap()
    bphi = nc.dram_tensor("bphi", [NHL * HD], F32, kind="ExternalInput").ap()
    RT = nc.dram_tensor("RT", [P, P], BF16, kind="ExternalInput").ap()
    out = nc.dram_tensor("out", [S, HID], BF16, kind="ExternalOutput").ap()

    xT_r = xT.rearrange("(ko p) s -> p ko s", p=P)
    Wq_r = Wq.rearrange("(ko p) m -> p ko m", p=P)
    Wkv_r = Wkv.rearrange("(ko p) m -> p ko m", p=P)
    Wphi_r = Wphi.rearrange("(ko p) m -> p ko m", p=P)
    Wo_r = Wo.rearrange("(h p) n -> p h n", p=P)
    css_sd_r = css_sd.rearrange("(t p) d -> p t d", p=P)
    bphi_r = bphi.rearrange("(h p) -> p h", p=P)
    out_r = out.rearrange("(t p) n -> p t n", p=P)

    from contextlib import ExitStack
    with tile.TileContext(nc) as tc, ExitStack() as es:
        # ---- pools ----
        res = es.enter_context(tc.tile_pool(name="res", bufs=1))        # residents
        wts = es.enter_context(tc.tile_pool(name="wts", bufs=2))        # big weights, shared slots
        xp = es.enter_context(tc.tile_pool(name="xp", bufs=3))          # xT chunks
        stream = es.enter_context(tc.tile_pool(name="stream", bufs=2))  # big per-chunk tiles
        stream3 = es.enter_context(tc.tile_pool(name="stream3", bufs=3))  # small per-chunk tiles
        small = es.enter_context(tc.tile_pool(name="small", bufs=2))    # tiny tiles
        # PSUM: 8 banks total. pa: all projections (kv/q/phi); pr: rope+result;
        # po: o_proj out; pmix: transposes/logits/softmax/outer (strictly sequential).
        pa = es.enter_context(tc.tile_pool(name="pa", bufs=3, space="PSUM"))
        pr = es.enter_context(tc.tile_pool(name="pr", bufs=2, space="PSUM"))
        po = es.enter_context(tc.tile_pool(name="po", bufs=2, space="PSUM"))
        pmix = es.enter_context(tc.tile_pool(name="pmix", bufs=1, space="PSUM"))

        # ---- residents / weights ----
        Wkv_sb = res.tile([P, KO, 2 * HD], BF16)
        nc.sync.dma_start(Wkv_sb[:, :KO // 4, :], Wkv_r[:, :KO // 4, :])
        RT_sb = res.tile([P, P], BF16)
        bphi_sb = res.tile([P, NHL], F32)
        Wq_sb = wts.tile([P, KO, NHL * HD], BF16, tag="big")

        ident_bf = res.tile([P, P], BF16)
        make_identity(nc, ident_bf[:])
        ident_f32 = res.tile([P, P], F32)
        make_identity(nc, ident_f32[:])
        ones_f32 = res.tile([P, 1], F32)
        nc.vector.memset(ones_f32[:], 1.0)
        onesr_f32 = res.tile([1, P], F32)
        nc.vector.memset(onesr_f32[:], 1.0)
        negr_f32 = res.tile([1, P], F32)
        nc.vector.memset(negr_f32[:], -1.0)

        QkT = res.tile([P, NHL, S], BF16)       # 32KB/part
        KkT = res.tile([P, S], BF16)            # 8KB/part
        Kk_sd = res.tile([P, NST, HD], BF16)    # 8KB/part
        v_sd = res.tile([P, NST, HD], BF16)     # 8KB/part
        qg_parts = res.tile([P, NHL, NCH], F32)
        qg_pre = res.tile([P, NHL], F32)
        outer_bf = res.tile([P, NHL, HD], BF16)
        alpha_bf = res.tile([P, NHL, NST], BF16)
        logits_sd = res.tile([P, NHL, NST], F32)

        # ================= phase A =================
        # Pending PE work that depends on a DVE chain; each entry is issued one
        # matmul-group later so the in-order PE queue never stalls.
        pend = []    # deferred ropes (SBUF-only inputs, depth 2)
        pend_t = []  # deferred KkT transposes (wait on a 4-engine kappa chain, depth 3)

        def flush_pending(min_depth=2, t_depth=3):
            # issue at most one deferred PE op per matmul group, transposes first
            if len(pend_t) >= t_depth:
                pend_t.pop(0)()
            elif len(pend) >= min_depth:
                pend.pop(0)()

        def make_transpose(stg):
            def f():
                pst = pmix.tile([P, P], BF16, tag="mix", name="pst")
                nc.tensor.transpose(pst[:], Kk_sd[:, stg, :], ident_bf[:])
                nc.any.tensor_copy(KkT[:, stg * P:(stg + 1) * P], pst[:])
            return f

        def flush_all_pending():
            while pend_t:
                pend_t.pop(0)()
            while pend:
                pend.pop(0)()

        def make_rope(h, c, qs, qc):
            # reads only SBUF tiles (qs, qc) + its own psum, so it can be
            # deferred without holding the projection PSUM slot
            def f():
                psr = pr.tile([P, CS], F32, tag="psr", name="psr")
                nc.tensor.matmul(psr[:], RT_sb[:], qs[:], start=True, stop=True)
                qro = stream.tile([P, CS], BF16, tag="qro", name="qro")
                nc.vector.tensor_add(qro[:], qc[:], psr[:])
                # kappa -> QkT, Qg partial folded in via accum_out
                mq = stream.tile([P, CS], BF16, tag="mq", name="mq")
                nc.gpsimd.tensor_scalar_min(mq[:], qro[:], 0.0)
                eq = stream.tile([P, CS], BF16, tag="eq", name="eq")
                nc.scalar.activation(eq[:], mq[:], mybir.ActivationFunctionType.Exp)
                nc.vector.scalar_tensor_tensor(
                    QkT[:, h, c * CS:(c + 1) * CS], qro[:], 0.0, eq[:],
                    mybir.AluOpType.max, mybir.AluOpType.add,
                    accum_out=qg_parts[:, h, c:c + 1])
            return f

        xt_c0_phaseC = [None]
        _hold = {}

        def load_xt_A(c, first):
            xt = xp.tile([P, KO, CS], BF16, tag="xt", name="xt")
            if first:
                # fine-grained first loads: the first kv matmuls start after just
                # Wkv(ko0-3) + xt(ko0-3, s0-255) have landed (~4us)
                nc.sync.dma_start(xt[:, :KO // 4, :CS // 2], xT_r[:, :KO // 4, :CS // 2])
                nc.sync.dma_start(Wkv_sb[:, KO // 4:KO // 2, :], Wkv_r[:, KO // 4:KO // 2, :])
                nc.sync.dma_start(xt[:, KO // 4:KO // 2, :CS // 2], xT_r[:, KO // 4:KO // 2, :CS // 2])
                nc.sync.dma_start(Wkv_sb[:, KO // 2:, :], Wkv_r[:, KO // 2:, :])
                nc.sync.dma_start(xt[:, KO // 2:, :CS // 2], xT_r[:, KO // 2:, :CS // 2])
            else:
                nc.sync.dma_start(xt[:, :, :CS // 2], xT_r[:, :, c * CS:c * CS + CS // 2])
            if c >= 2:
                # steady-state chunks: quarter-granular second half so the third
                # s-tile's data lands ~1.5us earlier (the queue has slack here)
                nc.sync.dma_start(xt[:, :, CS // 2:3 * CS // 4],
                                  xT_r[:, :, c * CS + CS // 2:c * CS + 3 * CS // 4])
                nc.sync.dma_start(xt[:, :, 3 * CS // 4:],
                                  xT_r[:, :, c * CS + 3 * CS // 4:(c + 1) * CS])
            else:
                nc.sync.dma_start(xt[:, :, CS // 2:], xT_r[:, :, c * CS + CS // 2:(c + 1) * CS])
            return xt

        for c in range(NCH):
            xt = load_xt_A(c, c == 0)
            # packed cos|sin tables: one DMA each for the k-rope and q-rope
            cssd = stream.tile([P, 4, HD], F32, tag="cossd", name="cssd")
            nc.sync.dma_start(cssd[:], css_sd_r[:, c * 4:(c + 1) * 4, :])
            csd = cssd[:, :, :64]
            ssd = cssd[:, :, 64:]
            if c == 0:
                nc.sync.dma_start(Wq_sb[:, :KO // 2, :], Wq_r[:, :KO // 2, :])
                nc.sync.dma_start(Wq_sb[:, KO // 2:, :], Wq_r[:, KO // 2:, :])
            cst2 = stream.tile([P, 2, CS], F32, tag="cosT", name="cst2")
            nc.sync.dma_start(cst2[:], csT2[:, :, c * CS:(c + 1) * CS])
            cs_t = cst2[:, 0, :]
            sn_t = cst2[:, 1, :]
            if c == 0:
                # deferred residents: not needed until the first rope / phase C
                nc.sync.dma_start(RT_sb[:], RT)
                nc.sync.dma_start(bphi_sb[:], bphi_r)
            if c == NCH - 1:
                # prefetch phase-C weights + first phase-C x chunk during the last
                # phase-A chunk's compute (interleaved so phi(c0,h0) can start on
                # the first halves of each)
                Wphi_t = wts.tile([P, KO, NHL * HD], BF16, tag="big")
                xtc0 = xp.tile([P, KO, CS], BF16, tag="xt", name="xt")
                nc.sync.dma_start(Wphi_t[:, :KO // 2, :], Wphi_r[:, :KO // 2, :])
                nc.sync.dma_start(xtc0[:, :, :CS // 2], xT_r[:, :, :CS // 2])
                nc.sync.dma_start(Wphi_t[:, KO // 2:, :], Wphi_r[:, KO // 2:, :])
                nc.sync.dma_start(xtc0[:, :, CS // 2:], xT_r[:, :, CS // 2:CS])
                xt_c0_phaseC[0] = xtc0
                _hold["Wphi"] = Wphi_t

            # ---- k + v for the 4 s-tiles of this chunk ----
            for st in range(4):
                stg = c * 4 + st
                pskv = pa.tile([P, 2 * HD], F32, tag="pa", name="pskv")
                for ko in range(KO):
                    nc.tensor.matmul(
                        pskv[:], xt[:, ko, st * P:(st + 1) * P], Wkv_sb[:, ko, :],
                        start=(ko == 0), stop=(ko == KO - 1))
                flush_pending()
                k_ps = pskv[:, :HD]
                nc.any.tensor_copy(v_sd[:, stg, :], pskv[:, HD:])
                # rope-k in [s,d]: rot on free halves (cos/sin halves along d are
                # equal, so broadcast the 64-wide tables)
                csb = csd[:, st, :].unsqueeze(1).broadcast_to([P, 2, 64])
                kr = stream3.tile([P, HD], F32, tag="kr", name="kr")
                nc.vector.tensor_mul(kr[:].rearrange("p (two d) -> p two d", two=2),
                                     k_ps.rearrange("p (two d) -> p two d", two=2), csb)
                ta = stream3.tile([P, 64], F32, tag="ta", name="ta")
                nc.vector.tensor_mul(ta[:], k_ps[:, 64:], ssd[:, st, :])
                nc.vector.tensor_sub(kr[:, :64], kr[:, :64], ta[:])
                tb = stream3.tile([P, 64], F32, tag="tb", name="tb")
                nc.vector.tensor_mul(tb[:], k_ps[:, :64], ssd[:, st, :])
                nc.vector.tensor_add(kr[:, 64:], kr[:, 64:], tb[:])
                # kappa
                mk = stream3.tile([P, HD], F32, tag="mk", name="mk")
                nc.gpsimd.tensor_scalar_min(mk[:], kr[:], 0.0)
                ek = stream3.tile([P, HD], F32, tag="ek", name="ek")
                nc.scalar.activation(ek[:], mk[:], mybir.ActivationFunctionType.Exp)
                nc.vector.scalar_tensor_tensor(
                    Kk_sd[:, stg, :], kr[:], 0.0, ek[:],
                    mybir.AluOpType.max, mybir.AluOpType.add)
                pend_t.append(make_transpose(stg))

            # ---- q heads ----
            for h in range(NHL):
                psq = pa.tile([P, CS], F32, tag="pa", name="psq")
                for ko in range(KO):
                    nc.tensor.matmul(
                        psq[:], Wq_sb[:, ko, h * HD:(h + 1) * HD], xt[:, ko, :],
                        start=(ko == 0), stop=(ko == KO - 1))
                flush_pending()
                # sin is 64-periodic over d, so rot(q)*sin == rot(q*sin):
                # multiply by sin BEFORE the rotation matmul (saves the psum copy)
                qs = stream3.tile([P, CS], BF16, tag="qbf", name="qs")
                nc.vector.tensor_mul(qs[:], psq[:], sn_t)
                qc = stream3.tile([P, CS], BF16, tag="qcbf", name="qc")
                nc.vector.tensor_mul(qc[:], psq[:], cs_t)
                pend.append(make_rope(h, c, qs, qc))
                if c == NCH - 1 and h == 1:
                    # all chunk<=6 ropes have now been flushed (the last one,
                    # rope(h3,c6), pops under this chunk's q1 group): the Qg
                    # partials for chunks 0..6 are final
                    for hh in range(NHL):
                        nc.vector.tensor_reduce(
                            qg_pre[:, hh:hh + 1], qg_parts[:, hh, :NCH - 1],
                            mybir.AxisListType.X, mybir.AluOpType.add)

        Wphi_sb = _hold["Wphi"]

        # ================= phase B + C =================
        Wo_sb = wts.tile([P, NHL, HID], BF16, tag="big")
        nc.scalar.dma_start(Wo_sb[:], Wo_r)
        # flush the remaining transposes/ropes now: their DVE chains overlap the
        # Wo load and phase C's first phi group, so Qg is ready for the logits
        flush_all_pending()

        # tiny sbuf tiles for the batched softmax
        qg_bf = small.tile([P, NHL], BF16, tag="qgbf")
        qg_f = small.tile([P, NHL], F32, tag="qgf")
        pmax4 = small.tile([P, NHL, 1], F32, tag="pmax4")
        gmax4 = small.tile([NHL, 1], F32, tag="gmax4")
        gms = small.tile([1, NHL], F32, tag="gms")
        ngm4 = small.tile([P, NHL], F32, tag="ngm4")
        e4 = small.tile([P, NHL, NST], F32, tag="e4", bufs=1)
        srow4 = small.tile([P, NHL], F32, tag="srow4")
        rcp4 = small.tile([NHL, 1], F32, tag="rcp4")
        rcs = small.tile([1, NHL], F32, tag="rcs")
        rcpb4 = small.tile([P, NHL], F32, tag="rcpb4")

        def issue_logits():
            # Qg finalize: add the last chunk's partial to the precomputed sum
            nc.vector.tensor_add(qg_f[:], qg_pre[:], qg_parts[:, :, NCH - 1])
            nc.vector.tensor_scalar_mul(qg_bf[:], qg_f[:], 1.0 / S)
            psl = pmix.tile([P, NST, NHL], F32, tag="mix", name="psl")
            for st in range(NST):
                nc.tensor.matmul(
                    psl[:, st, :], KkT[:, st * P:(st + 1) * P], qg_bf[:],
                    start=True, stop=True)
            nc.any.tensor_copy(logits_sd.rearrange("p h t -> p t h")[:], psl[:])
            nc.vector.tensor_reduce(
                pmax4[:], logits_sd[:], mybir.AxisListType.X, mybir.AluOpType.max)

        def issue_softmax1():
            # global max per head: transpose partial maxes, reduce, negate-broadcast
            pmt4 = pmix.tile([NHL, P], F32, tag="mix", name="pmt4")
            nc.tensor.transpose(pmt4[:], pmax4[:, :, 0], ident_f32[:])
            nc.vector.tensor_reduce(
                gmax4[:], pmt4[:], mybir.AxisListType.X, mybir.AluOpType.max)
            gmT = pmix.tile([1, NHL], F32, tag="mix", name="gmT")
            nc.tensor.transpose(gmT[:], gmax4[:], ident_f32[:NHL, :NHL])
            nc.vector.tensor_scalar_mul(gms[:], gmT[:], -1.0)
            pngm4 = pmix.tile([P, NHL], F32, tag="mix", name="pngm4")
            nc.tensor.matmul(pngm4[:], onesr_f32[:], gms[:], start=True, stop=True)
            nc.vector.tensor_copy(ngm4[:], pngm4[:])
            # e = exp(l - gmax) per head, row sums via accum_out
            for h in range(NHL):
                nc.scalar.activation(
                    e4[:, h, :], logits_sd[:, h, :], mybir.ActivationFunctionType.Exp,
                    bias=ngm4[:, h:h + 1], accum_out=srow4[:, h:h + 1])

        def issue_softmax2():
            # totals per head (fp32 matmul), reciprocal, broadcast, alpha = S * e / Z
            ptot4 = pmix.tile([NHL, 1], F32, tag="mix", name="ptot4")
            nc.tensor.matmul(ptot4[:], srow4[:], ones_f32[:], start=True, stop=True)
            nc.vector.reciprocal(rcp4[:], ptot4[:])
            rcT = pmix.tile([1, NHL], F32, tag="mix", name="rcT")
            nc.tensor.transpose(rcT[:], rcp4[:], ident_f32[:NHL, :NHL])
            nc.vector.tensor_copy(rcs[:], rcT[:])
            prc4 = pmix.tile([P, NHL], F32, tag="mix", name="prc4")
            nc.tensor.matmul(prc4[:], onesr_f32[:], rcs[:], start=True, stop=True)
            nc.vector.tensor_copy(rcpb4[:], prc4[:])
            nc.vector.scalar_tensor_tensor(
                alpha_bf[:], e4[:], float(S),
                rcpb4[:].unsqueeze(2).broadcast_to([P, NHL, NST]),
                mybir.AluOpType.mult, mybir.AluOpType.mult)
            nc.vector.tensor_copy(
                alpha2[:], alpha_bf[:].unsqueeze(3).broadcast_to([P, NHL, NST, 2]))

        QSTEP = NST // 4

        alpha2 = res.tile([P, NHL, NST, 2], BF16)

        def kka_mul(h, quarter):
            st0 = quarter * QSTEP
            sl = slice(st0, st0 + QSTEP)
            kka = stream3.tile([P, QSTEP, HD], BF16, tag="kka", name="kka", bufs=4)
            # keep every operand's last AP dim packed (stride 1) so the DVE
            # runs in its 2x mode: broadcast alpha over d via a middle dim,
            # with the duplicated pair in alpha2 supplying the packed tail
            ab = alpha2[:, h, sl, :].unsqueeze(2).broadcast_to([P, QSTEP, HD // 2, 2])
            nc.vector.tensor_mul(
                kka[:].rearrange("p t (j two) -> p t j two", two=2),
                Kk_sd[:, sl, :].rearrange("p t (j two) -> p t j two", two=2), ab)
            return kka

        kka_tiles = {}

        def prime_kka(n):
            blocks = [(h, q) for h in range(NHL) for q in range(4)]
            for b in blocks[:n]:
                if b not in kka_tiles:
                    kka_tiles[b] = kka_mul(*b)

        def issue_outer_all():
            # alpha (broadcast over d) * Kk in quarter-s blocks; the DVE muls run
            # one quarter ahead of the PE accumulation as a flat 16-stage pipeline
            blocks = [(h, q) for h in range(NHL) for q in range(4)]
            pso = None
            for i, (h, q) in enumerate(blocks):
                if i + 1 < len(blocks) and blocks[i + 1] not in kka_tiles:
                    kka_tiles[blocks[i + 1]] = kka_mul(*blocks[i + 1])
                if q == 0:
                    pso = pmix.tile([P, HD], F32, tag="mix", name="pso")
                kka = kka_tiles.pop((h, q))
                for st in range(QSTEP):
                    nc.tensor.matmul(pso[:], kka[:, st, :], v_sd[:, q * QSTEP + st, :],
                                     start=(q == 0 and st == 0),
                                     stop=(q == 3 and st == QSTEP - 1))
                if q == 3:
                    nc.any.tensor_copy(outer_bf[:, h, :], pso[:])

        copy_engines = [
            lambda dst, src: nc.vector.tensor_copy(dst, src),
            lambda dst, src: nc.scalar.copy(dst, src),
        ]
        _ctx_of = {}

        _ob4 = [None]

        def oproj_group(c, st, n, pool, tag):
            stg = c * 4 + st
            pso2 = pool.tile([P, 512], F32, tag=tag, name="pso2")
            if c == NCH - 1 and st == 3 and n >= 2:
                # two half-column accumulation groups: the copy of the first half
                # starts while the second half is still accumulating
                for ch in range(2):
                    csl = slice(ch * 256, (ch + 1) * 256)
                    for h in range(NHL):
                        nc.tensor.matmul(
                            pso2[:, csl], _ctx_of[c][:, h, st * P:(st + 1) * P],
                            Wo_sb[:, h, n * 512 + ch * 256:n * 512 + (ch + 1) * 256],
                            start=(h == 0), stop=(h == NHL - 1))
                    copy_engines[ch](_ob4[0][:, n * 512 + ch * 256:n * 512 + (ch + 1) * 256],
                                     pso2[:, csl])
                nc.sync.dma_start(out_r[:, stg, n * 512:(n + 1) * 512],
                                  _ob4[0][:, n * 512:(n + 1) * 512])
                return
            for h in range(NHL):
                nc.tensor.matmul(
                    pso2[:], _ctx_of[c][:, h, st * P:(st + 1) * P],
                    Wo_sb[:, h, n * 512:(n + 1) * 512],
                    start=(h == 0), stop=(h == NHL - 1))
            if n == 0:
                _ob4[0] = stream.tile([P, HID], BF16, tag="ob", name="ob", bufs=2)
            copy_engines[(st + n) % 2](_ob4[0][:, n * 512:(n + 1) * 512], pso2[:])
            if c == NCH - 1 and st == 3:
                # drain the final tile at decreasing granularity so the last
                # DMA only waits for the last copy (descriptor cost is 625ns)
                if n == 1:
                    nc.sync.dma_start(out_r[:, stg, :1024], _ob4[0][:, :1024])
                elif n >= 2:
                    nc.sync.dma_start(out_r[:, stg, n * 512:(n + 1) * 512],
                                      _ob4[0][:, n * 512:(n + 1) * 512])
            elif c == NCH - 1 and st == 2:
                if n == 1:
                    nc.sync.dma_start(out_r[:, stg, :1024], _ob4[0][:, :1024])
                elif n == 3:
                    nc.sync.dma_start(out_r[:, stg, 1024:], _ob4[0][:, 1024:])
            elif n == 3:
                # middle chunks drain on the second HWDGE queue so the out
                # writes never delay the next x-chunk prefetch
                (nc.scalar if c < NCH - 1 else nc.sync).dma_start(
                    out_r[:, stg, :], _ob4[0][:])

        def issue_oproj(c, groups=range(16)):
            for g in groups:
                oproj_group(c, g // 4, g % 4, po, "psout")

        for c in range(NCH):
            if c == 0:
                xt = xt_c0_phaseC[0]
            else:
                xt = xp.tile([P, KO, CS], BF16, tag="xt", name="xt")
                nc.sync.dma_start(xt[:, :, :CS // 2], xT_r[:, :, c * CS:c * CS + CS // 2])
                nc.sync.dma_start(xt[:, :, CS // 2:], xT_r[:, :, c * CS + CS // 2:(c + 1) * CS])
            ctx_bf = stream.tile([P, NHL, CS], BF16, tag="ctx", name="ctx_bf")
            _ctx_of[c] = ctx_bf
            for h in range(NHL):
                psp = pa.tile([P, CS], F32, tag="pa", name="psp")
                if c == 0 and h == 0:
                    # split over s-halves: starts as soon as the first halves of
                    # Wphi and x(c0) have landed
                    for sh in range(2):
                        ssl = slice(sh * (CS // 2), (sh + 1) * (CS // 2))
                        for ko in range(KO):
                            nc.tensor.matmul(
                                psp[:, ssl], Wphi_sb[:, ko, h * HD:(h + 1) * HD],
                                xt[:, ko, ssl], start=(ko == 0), stop=(ko == KO - 1))
                else:
                    for ko in range(KO):
                        nc.tensor.matmul(
                            psp[:], Wphi_sb[:, ko, h * HD:(h + 1) * HD], xt[:, ko, :],
                            start=(ko == 0), stop=(ko == KO - 1))
                if c == 0:
                    # pending ropes, logits, softmax ride inside chunk 0's phi groups
                    if h == 0:
                        flush_all_pending()
                    elif h == 1:
                        issue_logits()
                    elif h == 2:
                        issue_softmax1()
                        issue_softmax2()
                        prime_kka(3)
                else:
                    # previous chunk's o_proj rides between this chunk's phi
                    # head-groups so its psum->sbuf copies never stall PE
                    issue_oproj(c - 1, range(h * 4, h * 4 + 4))
                phiT = stream.tile([P, CS], BF16, tag="phiT", name="phiT", bufs=4)
                nc.scalar.activation(phiT[:], psp[:], mybir.ActivationFunctionType.Identity,
                                     bias=bphi_sb[:, h:h + 1])
                if c > 0:
                    psr = pr.tile([P, CS], F32, tag="psr", name="psr")
                    nc.tensor.matmul(psr[:], outer_bf[:, h, :],
                                     QkT[:, h, c * CS:(c + 1) * CS], start=True, stop=True)
                    nc.vector.tensor_mul(ctx_bf[:, h, :], phiT[:], psr[:])
                else:
                    _phiT_c0 = _ctx_of.setdefault("phiT_c0", [])
                    _phiT_c0.append(phiT)
            if c == 0:
                issue_outer_all()
                for h in range(NHL):
                    psr = pr.tile([P, CS], F32, tag="psr", name="psr")
                    nc.tensor.matmul(psr[:], outer_bf[:, h, :],
                                     QkT[:, h, :CS], start=True, stop=True)
                    nc.vector.tensor_mul(ctx_bf[:, h, :], _ctx_of["phiT_c0"][h][:], psr[:])
        # final chunk's o_proj: alternate psum between po and the now-idle pa pool
        # for deeper buffering (the copy latency never blocks the matmuls)
        for g in range(16):
            oproj_group(NCH - 1, g // 4, g % 4, (po, pa)[g % 2], ("psout", "pa")[g % 2])

    nc.compile()
    return nc


def _host_prep(hidden_states, position_ids, Wq, Wk, Wv, Wo, Wphi, bphi):
    B = hidden_states.shape[0]
    # rope tables (match reference fp32 math)
    inv_freq = (1.0 / (ROPE_THETA ** (np.arange(0, HD, 2, dtype=np.float32) / HD))).astype(np.float32)
    in_maps = []
    Rm = np.zeros((P, P), dtype=np.float32)
    Rm[np.arange(64), np.arange(64) + 64] = -1.0
    Rm[np.arange(64) + 64, np.arange(64)] = 1.0
    RT_np = np.ascontiguousarray(Rm.T).astype(BF)
    for b in range(B):
        freqs = position_ids[b].astype(np.float32)[:, None] * inv_freq[None, :]
        emb = np.concatenate([freqs, freqs], axis=1)          # [S, 128]
        cos_b = np.cos(emb).astype(np.float32)
        sin_b = np.sin(emb).astype(np.float32)
        xT_b = np.ascontiguousarray(hidden_states[b].T).astype(BF)
        cosT_b = np.ascontiguousarray(cos_b.T)
        sinT_b = np.ascontiguousarray(sin_b.T)
        for g in range(4):
            sl4 = slice(g * 512, (g + 1) * 512)
            sl1 = slice(g * 128, (g + 1) * 128)
            in_maps.append({
                "xT": xT_b,
                "csT2": np.ascontiguousarray(np.stack([cosT_b, sinT_b], axis=1)),
                "css_sd": np.ascontiguousarray(
                    np.concatenate([cos_b[:, :64], sin_b[:, :64]], axis=1)),
                "Wq": np.ascontiguousarray(Wq[:, sl4]).astype(BF),
                "Wkv": np.ascontiguousarray(
                    np.concatenate([Wk[:, sl1], Wv[:, sl1]], axis=1)).astype(BF),
                "Wphi": np.ascontiguousarray(Wphi[:, sl4]).astype(BF),
                "Wo": np.ascontiguousarray(Wo[sl4, :]).astype(BF),
                "bphi": np.ascontiguousarray(bphi[sl4]).astype(np.float32),
                "RT": RT_np,
            })
    return in_maps


def kernel(hidden_states, position_ids, Wq, Wk, Wv, Wo, Wphi, bphi, _trace=False):
    if "nc" not in _CACHE:
        _CACHE["nc"] = _build()
    nc = _CACHE["nc"]
    in_maps = _host_prep(np.asarray(hidden_states), np.asarray(position_ids),
                         np.asarray(Wq), np.asarray(Wk), np.asarray(Wv),
                         np.asarray(Wo), np.asarray(Wphi), np.asarray(bphi))
    res = run_bass_kernel_spmd(nc, in_maps, list(range(8)), trace=_trace)
    _CACHE["last_res"] = res
    B = hidden_states.shape[0]
    out = np.empty((B, S, HID), dtype=np.float32)
    for b in range(B):
        acc = res.results[b * 4 + 0]["out"].astype(np.float32)
        for g in range(1, 4):
            acc = acc + res.results[b * 4 + g]["out"].astype(np.float32)
        out[b] = acc
    return out


# revision 84
# speedup vs baseline: 1.0004x; 1.0004x over previous
"""Trainium2 Bass kernel for LlamaRALAAttention (B=2, S=4096, HID=2048, NH=16, NKV=4, HD=128).

Sharding: 8 cores = DP(batch=2) x TP(kv-head groups=4). Core c handles batch c//4,
kv group c%4 (4 q heads + 1 kv head). Softmax/mean over S stay core-local.
o_proj partials are written bf16 and summed on host (the only cross-core reduction).

Pipeline (per core, "everything transposed" layout):
  xT [HID,S] host-pretransposed, bf16. Projections stream xT chunks as moving operand.
  q path in [d,s] layout: q^T = Wq_h^T @ xT, RoPE via R-matmul + cos/sin mults,
    kappa=exp(min(x,0))+max(x,0) -> QkT (bf16, resident); Qg partial folded into the
    kappa op via accum_out.
  k/v path in [s,d] layout: lhsT=xT tile (stationary), rhs=[Wk|Wv]; RoPE on free dim;
    kappa -> Kk_sd, v_sd (bf16, resident). KkT via PE transpose.
  All DVE-fed PE ops (KkT transposes, q-rope matmuls) go through a pending FIFO and are
  issued one matmul-group late, so the in-order PE queue never waits on a DVE chain.
  Logits: per-s-tile matvecs batched over the 4 heads into one PSUM bank; softmax
  (exact global max) batched over heads and interleaved into the first phi chunk;
  outer = (alpha*Kk)^T @ v with alpha applied via broadcast muls; result^T: lhsT=outer,
  rhs=QkT; ctx^T = phiT * result^T; o_proj one chunk behind phi/result so the ctx DVE
  muls never stall PE.
"""

import sys

sys.path.insert(0, "/opt/trn_rl_repo")

import numpy as np
import ml_dtypes

import concourse.bass as bass
import concourse.mybir as mybir
import concourse.tile as tile
from concourse import bacc
from concourse.bass_utils import run_bass_kernel_spmd
from concourse.masks import make_identity

P = 128
S = 4096
HID = 2048
HD = 128
NHL = 4            # q heads per core
KO = HID // P      # 16 contraction subtiles
CS = 512           # token chunk size
NCH = S // CS      # 8 chunks
NST = S // P       # 32 s-tiles
HSTEP = NST // 2
ROPE_THETA = 10000.0

F32 = mybir.dt.float32
BF16 = mybir.dt.bfloat16
BF = ml_dtypes.bfloat16

_CACHE = {}


def _build():
    nc = bacc.Bacc("TRN2", target_bir_lowering=False, debug=False, num_devices=8)

    xT = nc.dram_tensor("xT", [HID, S], BF16, kind="ExternalInput").ap()
    csT2 = nc.dram_tensor("csT2", [P, 2, S], F32, kind="ExternalInput").ap()
    css_sd = nc.dram_tensor("css_sd", [S, HD], F32, kind="ExternalInput").ap()
    Wq = nc.dram_tensor("Wq", [HID, NHL * HD], BF16, kind="ExternalInput").ap()
    Wkv = nc.dram_tensor("Wkv", [HID, 2 * HD], BF16, kind="ExternalInput").ap()
    Wphi = nc.dram_tensor("Wphi", [HID, NHL * HD], BF16, kind="ExternalInput").ap()
    Wo = nc.dram_tensor("Wo", [NHL * HD, HID], BF16, kind="ExternalInput").ap()
    bphi = nc.dram_tensor("bphi", [NHL * HD], F32, kind="ExternalInput").ap()
    RT = nc.dram_tensor("RT", [P, P], BF16, kind="ExternalInput").ap()
    out = nc.dram_tensor("out", [S, HID], BF16, kind="ExternalOutput").ap()

    xT_r = xT.rearrange("(ko p) s -> p ko s", p=P)
    Wq_r = Wq.rearrange("(ko p) m -> p ko m", p=P)
    Wkv_r = Wkv.rearrange("(ko p) m -> p ko m", p=P)
    Wphi_r = Wphi.rearrange("(ko p) m -> p ko m", p=P)
    Wo_r = Wo.rearrange("(h p) n -> p h n", p=P)
    css_sd_r = css_sd.rearrange("(t p) d -> p t d", p=P)
    bphi_r = bphi.rearrange("(h p) -> p h", p=P)
    out_r = out.rearrange("(t p) n -> p t n", p=P)

    from contextlib import ExitStack
    with tile.TileContext(nc) as tc, ExitStack() as es:
        # ---- pools ----
        res = es.enter_context(tc.tile_pool(name="res", bufs=1))        # residents
        wts = es.enter_context(tc.tile_pool(name="wts", bufs=2))        # big weights, shared slots
        xp = es.enter_context(tc.tile_pool(name="xp", bufs=3))          # xT chunks
        stream = es.enter_context(tc.tile_pool(name="stream", bufs=2))  # big per-chunk tiles
        stream3 = es.enter_context(tc.tile_pool(name="stream3", bufs=3))  # small per-chunk tiles
        small = es.enter_context(tc.tile_pool(name="small", bufs=2))    # tiny tiles
        # PSUM: 8 banks total. pa: all projections (kv/q/phi); pr: rope+result;
        # po: o_proj out; pmix: transposes/logits/softmax/outer (strictly sequential).
        pa = es.enter_context(tc.tile_pool(name="pa", bufs=3, space="PSUM"))
        pr = es.enter_context(tc.tile_pool(name="pr", bufs=2, space="PSUM"))
        po = es.enter_context(tc.tile_pool(name="po", bufs=2, space="PSUM"))
        pmix = es.enter_context(tc.tile_pool(name="pmix", bufs=1, space="PSUM"))

        # ---- residents / weights ----
        Wkv_sb = res.tile([P, KO, 2 * HD], BF16)
        nc.sync.dma_start(Wkv_sb[:, :KO // 4, :], Wkv_r[:, :KO // 4, :])
        RT_sb = res.tile([P, P], BF16)
        bphi_sb = res.tile([P, NHL], F32)
        Wq_sb = wts.tile([P, KO, NHL * HD], BF16, tag="big")

        ident_bf = res.tile([P, P], BF16)
        make_identity(nc, ident_bf[:])
        ident_f32 = res.tile([P, P], F32)
        make_identity(nc, ident_f32[:])
        ones_f32 = res.tile([P, 1], F32)
        nc.vector.memset(ones_f32[:], 1.0)
        onesr_f32 = res.tile([1, P], F32)
        nc.vector.memset(onesr_f32[:], 1.0)
        negr_f32 = res.tile([1, P], F32)
        nc.vector.memset(negr_f32[:], -1.0)

        QkT = res.tile([P, NHL, S], BF16)       # 32KB/part
        KkT = res.tile([P, S], BF16)            # 8KB/part
        Kk_sd = res.tile([P, NST, HD], BF16)    # 8KB/part
        v_sd = res.tile([P, NST, HD], BF16)     # 8KB/part
        qg_parts = res.tile([P, NHL, NCH], F32)
        qg_pre = res.tile([P, NHL], F32)
        outer_bf = res.tile([P, NHL, HD], BF16)
        alpha_bf = res.tile([P, NHL, NST], BF16)
        logits_sd = res.tile([P, NHL, NST], F32)

        # ================= phase A =================
        # Pending PE work that depends on a DVE chain; each entry is issued one
        # matmul-group later so the in-order PE queue never stalls.
        pend = []    # deferred ropes (SBUF-only inputs, depth 2)
        pend_t = []  # deferred KkT transposes (wait on a 4-engine kappa chain, depth 3)

        def flush_pending(min_depth=2, t_depth=3):
            # issue at most one deferred PE op per matmul group, transposes first
            if len(pend_t) >= t_depth:
                pend_t.pop(0)()
            elif len(pend) >= min_depth:
                pend.pop(0)()

        def make_transpose(stg):
            def f():
                pst = pmix.tile([P, P], BF16, tag="mix", name="pst")
                nc.tensor.transpose(pst[:], Kk_sd[:, stg, :], ident_bf[:])
                nc.any.tensor_copy(KkT[:, stg * P:(stg + 1) * P], pst[:])
            return f

        def flush_all_pending():
            while pend_t:
                pend_t.pop(0)()
            while pend:
                pend.pop(0)()

        def make_rope(h, c, qs, qc):
            # reads only SBUF tiles (qs, qc) + its own psum, so it can be
            # deferred without holding the projection PSUM slot
            def f():
                psr = pr.tile([P, CS], F32, tag="psr", name="psr")
                nc.tensor.matmul(psr[:], RT_sb[:], qs[:], start=True, stop=True)
                qro = stream.tile([P, CS], BF16, tag="qro", name="qro")
                nc.vector.tensor_add(qro[:], qc[:], psr[:])
                # kappa -> QkT, Qg partial folded in via accum_out
                mq = stream.tile([P, CS], BF16, tag="mq", name="mq")
                nc.gpsimd.tensor_scalar_min(mq[:], qro[:], 0.0)
                eq = stream.tile([P, CS], BF16, tag="eq", name="eq")
                nc.scalar.activation(eq[:], mq[:], mybir.ActivationFunctionType.Exp)
                nc.vector.scalar_tensor_tensor(
                    QkT[:, h, c * CS:(c + 1) * CS], qro[:], 0.0, eq[:],
                    mybir.AluOpType.max, mybir.AluOpType.add,
                    accum_out=qg_parts[:, h, c:c + 1])
            return f

        xt_c0_phaseC = [None]
        _hold = {}

        def load_xt_A(c, first):
            xt = xp.tile([P, KO, CS], BF16, tag="xt", name="xt")
            if first:
                # fine-grained first loads: the first kv matmuls start after just
                # Wkv(ko0-3) + xt(ko0-3, s0-255) have landed (~4us)
                nc.sync.dma_start(xt[:, :KO // 4, :CS // 2], xT_r[:, :KO // 4, :CS // 2])
                nc.sync.dma_start(Wkv_sb[:, KO // 4:KO // 2, :], Wkv_r[:, KO // 4:KO // 2, :])
                nc.sync.dma_start(xt[:, KO // 4:KO // 2, :CS // 2], xT_r[:, KO // 4:KO // 2, :CS // 2])
                nc.sync.dma_start(Wkv_sb[:, KO // 2:, :], Wkv_r[:, KO // 2:, :])
                nc.sync.dma_start(xt[:, KO // 2:, :CS // 2], xT_r[:, KO // 2:, :CS // 2])
            else:
                nc.sync.dma_start(xt[:, :, :CS // 2], xT_r[:, :, c * CS:c * CS + CS // 2])
            nc.sync.dma_start(xt[:, :, CS // 2:], xT_r[:, :, c * CS + CS // 2:(c + 1) * CS])
            return xt

        for c in range(NCH):
            xt = load_xt_A(c, c == 0)
            # packed cos|sin tables: one DMA each for the k-rope and q-rope
            cssd = stream.tile([P, 4, HD], F32, tag="cossd", name="cssd")
            nc.sync.dma_start(cssd[:], css_sd_r[:, c * 4:(c + 1) * 4, :])
            csd = cssd[:, :, :64]
            ssd = cssd[:, :, 64:]
            if c == 0:
                nc.sync.dma_start(Wq_sb[:, :KO // 2, :], Wq_r[:, :KO // 2, :])
                nc.sync.dma_start(Wq_sb[:, KO // 2:, :], Wq_r[:, KO // 2:, :])
            cst2 = stream.tile([P, 2, CS], F32, tag="cosT", name="cst2")
            nc.sync.dma_start(cst2[:], csT2[:, :, c * CS:(c + 1) * CS])
            cs_t = cst2[:, 0, :]
            sn_t = cst2[:, 1, :]
            if c == 0:
                # deferred residents: not needed until the first rope / phase C
                nc.sync.dma_start(RT_sb[:], RT)
                nc.sync.dma_start(bphi_sb[:], bphi_r)
            if c == NCH - 1:
                # prefetch phase-C weights + first phase-C x chunk during the last
                # phase-A chunk's compute (interleaved so phi(c0,h0) can start on
                # the first halves of each)
                Wphi_t = wts.tile([P, KO, NHL * HD], BF16, tag="big")
                xtc0 = xp.tile([P, KO, CS], BF16, tag="xt", name="xt")
                nc.sync.dma_start(Wphi_t[:, :KO // 2, :], Wphi_r[:, :KO // 2, :])
                nc.sync.dma_start(xtc0[:, :, :CS // 2], xT_r[:, :, :CS // 2])
                nc.sync.dma_start(Wphi_t[:, KO // 2:, :], Wphi_r[:, KO // 2:, :])
                nc.sync.dma_start(xtc0[:, :, CS // 2:], xT_r[:, :, CS // 2:CS])
                xt_c0_phaseC[0] = xtc0
                _hold["Wphi"] = Wphi_t

            # ---- k + v for the 4 s-tiles of this chunk ----
            for st in range(4):
                stg = c * 4 + st
                pskv = pa.tile([P, 2 * HD], F32, tag="pa", name="pskv")
                for ko in range(KO):
                    nc.tensor.matmul(
                        pskv[:], xt[:, ko, st * P:(st + 1) * P], Wkv_sb[:, ko, :],
                        start=(ko == 0), stop=(ko == KO - 1))
                flush_pending()
                k_ps = pskv[:, :HD]
                nc.any.tensor_copy(v_sd[:, stg, :], pskv[:, HD:])
                # rope-k in [s,d]: rot on free halves (cos/sin halves along d are
                # equal, so broadcast the 64-wide tables)
                csb = csd[:, st, :].unsqueeze(1).broadcast_to([P, 2, 64])
                kr = stream3.tile([P, HD], F32, tag="kr", name="kr")
                nc.vector.tensor_mul(kr[:].rearrange("p (two d) -> p two d", two=2),
                                     k_ps.rearrange("p (two d) -> p two d", two=2), csb)
                ta = stream3.tile([P, 64], F32, tag="ta", name="ta")
                nc.vector.tensor_mul(ta[:], k_ps[:, 64:], ssd[:, st, :])
                nc.vector.tensor_sub(kr[:, :64], kr[:, :64], ta[:])
                tb = stream3.tile([P, 64], F32, tag="tb", name="tb")
                nc.vector.tensor_mul(tb[:], k_ps[:, :64], ssd[:, st, :])
                nc.vector.tensor_add(kr[:, 64:], kr[:, 64:], tb[:])
                # kappa
                mk = stream3.tile([P, HD], F32, tag="mk", name="mk")
                nc.gpsimd.tensor_scalar_min(mk[:], kr[:], 0.0)
                ek = stream3.tile([P, HD], F32, tag="ek", name="ek")
                nc.scalar.activation(ek[:], mk[:], mybir.ActivationFunctionType.Exp)
                nc.vector.scalar_tensor_tensor(
                    Kk_sd[:, stg, :], kr[:], 0.0, ek[:],
                    mybir.AluOpType.max, mybir.AluOpType.add)
                pend_t.append(make_transpose(stg))

            # ---- q heads ----
            for h in range(NHL):
                psq = pa.tile([P, CS], F32, tag="pa", name="psq")
                for ko in range(KO):
                    nc.tensor.matmul(
                        psq[:], Wq_sb[:, ko, h * HD:(h + 1) * HD], xt[:, ko, :],
                        start=(ko == 0), stop=(ko == KO - 1))
                flush_pending()
                # sin is 64-periodic over d, so rot(q)*sin == rot(q*sin):
                # multiply by sin BEFORE the rotation matmul (saves the psum copy)
                qs = stream3.tile([P, CS], BF16, tag="qbf", name="qs")
                nc.vector.tensor_mul(qs[:], psq[:], sn_t)
                qc = stream3.tile([P, CS], BF16, tag="qcbf", name="qc")
                nc.vector.tensor_mul(qc[:], psq[:], cs_t)
                pend.append(make_rope(h, c, qs, qc))
                if c == NCH - 1 and h == 1:
                    # all chunk<=6 ropes have now been flushed (the last one,
                    # rope(h3,c6), pops under this chunk's q1 group): the Qg
                    # partials for chunks 0..6 are final
                    for hh in range(NHL):
                        nc.vector.tensor_reduce(
                            qg_pre[:, hh:hh + 1], qg_parts[:, hh, :NCH - 1],
                            mybir.AxisListType.X, mybir.AluOpType.add)

        Wphi_sb = _hold["Wphi"]

        # ================= phase B + C =================
        Wo_sb = wts.tile([P, NHL, HID], BF16, tag="big")
        nc.scalar.dma_start(Wo_sb[:], Wo_r)
        # flush the remaining transposes/ropes now: their DVE chains overlap the
        # Wo load and phase C's first phi group, so Qg is ready for the logits
        flush_all_pending()

        # tiny sbuf tiles for the batched softmax
        qg_bf = small.tile([P, NHL], BF16, tag="qgbf")
        qg_f = small.tile([P, NHL], F32, tag="qgf")
        pmax4 = small.tile([P, NHL, 1], F32, tag="pmax4")
        gmax4 = small.tile([NHL, 1], F32, tag="gmax4")
        gms = small.tile([1, NHL], F32, tag="gms")
        ngm4 = small.tile([P, NHL], F32, tag="ngm4")
        e4 = small.tile([P, NHL, NST], F32, tag="e4", bufs=1)
        srow4 = small.tile([P, NHL], F32, tag="srow4")
        rcp4 = small.tile([NHL, 1], F32, tag="rcp4")
        rcs = small.tile([1, NHL], F32, tag="rcs")
        rcpb4 = small.tile([P, NHL], F32, tag="rcpb4")

        def issue_logits():
            # Qg finalize: add the last chunk's partial to the precomputed sum
            nc.vector.tensor_add(qg_f[:], qg_pre[:], qg_parts[:, :, NCH - 1])
            nc.vector.tensor_scalar_mul(qg_bf[:], qg_f[:], 1.0 / S)
            psl = pmix.tile([P, NST, NHL], F32, tag="mix", name="psl")
            for st in range(NST):
                nc.tensor.matmul(
                    psl[:, st, :], KkT[:, st * P:(st + 1) * P], qg_bf[:],
                    start=True, stop=True)
            nc.any.tensor_copy(logits_sd.rearrange("p h t -> p t h")[:], psl[:])
            nc.vector.tensor_reduce(
                pmax4[:], logits_sd[:], mybir.AxisListType.X, mybir.AluOpType.max)

        def issue_softmax1():
            # global max per head: transpose partial maxes, reduce, negate-broadcast
            pmt4 = pmix.tile([NHL, P], F32, tag="mix", name="pmt4")
            nc.tensor.transpose(pmt4[:], pmax4[:, :, 0], ident_f32[:])
            nc.vector.tensor_reduce(
                gmax4[:], pmt4[:], mybir.AxisListType.X, mybir.AluOpType.max)
            gmT = pmix.tile([1, NHL], F32, tag="mix", name="gmT")
            nc.tensor.transpose(gmT[:], gmax4[:], ident_f32[:NHL, :NHL])
            nc.vector.tensor_scalar_mul(gms[:], gmT[:], -1.0)
            pngm4 = pmix.tile([P, NHL], F32, tag="mix", name="pngm4")
            nc.tensor.matmul(pngm4[:], onesr_f32[:], gms[:], start=True, stop=True)
            nc.vector.tensor_copy(ngm4[:], pngm4[:])
            # e = exp(l - gmax) per head, row sums via accum_out
            for h in range(NHL):
                nc.scalar.activation(
                    e4[:, h, :], logits_sd[:, h, :], mybir.ActivationFunctionType.Exp,
                    bias=ngm4[:, h:h + 1], accum_out=srow4[:, h:h + 1])

        def issue_softmax2():
            # totals per head (fp32 matmul), reciprocal, broadcast, alpha = S * e / Z
            ptot4 = pmix.tile([NHL, 1], F32, tag="mix", name="ptot4")
            nc.tensor.matmul(ptot4[:], srow4[:], ones_f32[:], start=True, stop=True)
            nc.vector.reciprocal(rcp4[:], ptot4[:])
            rcT = pmix.tile([1, NHL], F32, tag="mix", name="rcT")
            nc.tensor.transpose(rcT[:], rcp4[:], ident_f32[:NHL, :NHL])
            nc.vector.tensor_copy(rcs[:], rcT[:])
            prc4 = pmix.tile([P, NHL], F32, tag="mix", name="prc4")
            nc.tensor.matmul(prc4[:], onesr_f32[:], rcs[:], start=True, stop=True)
            nc.vector.tensor_copy(rcpb4[:], prc4[:])
            nc.vector.scalar_tensor_tensor(
                alpha_bf[:], e4[:], float(S),
                rcpb4[:].unsqueeze(2).broadcast_to([P, NHL, NST]),
                mybir.AluOpType.mult, mybir.AluOpType.mult)
            nc.vector.tensor_copy(
                alpha2[:], alpha_bf[:].unsqueeze(3).broadcast_to([P, NHL, NST, 2]))

        QSTEP = NST // 4

        alpha2 = res.tile([P, NHL, NST, 2], BF16)

        def kka_mul(h, quarter):
            st0 = quarter * QSTEP
            sl = slice(st0, st0 + QSTEP)
            kka = stream3.tile([P, QSTEP, HD], BF16, tag="kka", name="kka", bufs=4)
            # keep every operand's last AP dim packed (stride 1) so the DVE
            # runs in its 2x mode: broadcast alpha over d via a middle dim,
            # with the duplicated pair in alpha2 supplying the packed tail
            ab = alpha2[:, h, sl, :].unsqueeze(2).broadcast_to([P, QSTEP, HD // 2, 2])
            nc.vector.tensor_mul(
                kka[:].rearrange("p t (j two) -> p t j two", two=2),
                Kk_sd[:, sl, :].rearrange("p t (j two) -> p t j two", two=2), ab)
            return kka

        kka_tiles = {}

        def prime_kka(n):
            blocks = [(h, q) for h in range(NHL) for q in range(4)]
            for b in blocks[:n]:
                if b not in kka_tiles:
                    kka_tiles[b] = kka_mul(*b)

        def issue_outer_all():
            # alpha (broadcast over d) * Kk in quarter-s blocks; the DVE muls run
            # one quarter ahead of the PE accumulation as a flat 16-stage pipeline
            blocks = [(h, q) for h in range(NHL) for q in range(4)]
            pso = None
            for i, (h, q) in enumerate(blocks):
                if i + 1 < len(blocks) and blocks[i + 1] not in kka_tiles:
                    kka_tiles[blocks[i + 1]] = kka_mul(*blocks[i + 1])
                if q == 0:
                    pso = pmix.tile([P, HD], F32, tag="mix", name="pso")
                kka = kka_tiles.pop((h, q))
                for st in range(QSTEP):
                    nc.tensor.matmul(pso[:], kka[:, st, :], v_sd[:, q * QSTEP + st, :],
                                     start=(q == 0 and st == 0),
                                     stop=(q == 3 and st == QSTEP - 1))
                if q == 3:
                    nc.any.tensor_copy(outer_bf[:, h, :], pso[:])

        copy_engines = [
            lambda dst, src: nc.vector.tensor_copy(dst, src),
            lambda dst, src: nc.scalar.copy(dst, src),
        ]
        _ctx_of = {}

        _ob4 = [None]

        def oproj_group(c, st, n, pool, tag):
            stg = c * 4 + st
            pso2 = pool.tile([P, 512], F32, tag=tag, name="pso2")
            if c == NCH - 1 and st == 3 and n >= 2:
                # two half-column accumulation groups: the copy of the first half
                # starts while the second half is still accumulating
                for ch in range(2):
                    csl = slice(ch * 256, (ch + 1) * 256)
                    for h in range(NHL):
                        nc.tensor.matmul(
                            pso2[:, csl], _ctx_of[c][:, h, st * P:(st + 1) * P],
                            Wo_sb[:, h, n * 512 + ch * 256:n * 512 + (ch + 1) * 256],
                            start=(h == 0), stop=(h == NHL - 1))
                    copy_engines[ch](_ob4[0][:, n * 512 + ch * 256:n * 512 + (ch + 1) * 256],
                                     pso2[:, csl])
                nc.sync.dma_start(out_r[:, stg, n * 512:(n + 1) * 512],
                                  _ob4[0][:, n * 512:(n + 1) * 512])
                return
            for h in range(NHL):
                nc.tensor.matmul(
                    pso2[:], _ctx_of[c][:, h, st * P:(st + 1) * P],
                    Wo_sb[:, h, n * 512:(n + 1) * 512],
                    start=(h == 0), stop=(h == NHL - 1))
            if n == 0:
                _ob4[0] = stream.tile([P, HID], BF16, tag="ob", name="ob", bufs=2)
            copy_engines[(st + n) % 2](_ob4[0][:, n * 512:(n + 1) * 512], pso2[:])
            if c == NCH - 1 and st == 3:
                # drain the final tile at decreasing granularity so the last
                # DMA only waits for the last copy (descriptor cost is 625ns)
                if n == 1:
                    nc.sync.dma_start(out_r[:, stg, :1024], _ob4[0][:, :1024])
                elif n >= 2:
                    nc.sync.dma_start(out_r[:, stg, n * 512:(n + 1) * 512],
                                      _ob4[0][:, n * 512:(n + 1) * 512])
            elif c == NCH - 1 and st == 2:
                if n == 1:
                    nc.sync.dma_start(out_r[:, stg, :1024], _ob4[0][:, :1024])
                elif n == 3:
                    nc.sync.dma_start(out_r[:, stg, 1024:], _ob4[0][:, 1024:])
            elif n == 3:
                # middle chunks drain on the second HWDGE queue so the out
                # writes never delay the next x-chunk prefetch
                (nc.scalar if c < NCH - 1 else nc.sync).dma_start(
                    out_r[:, stg, :], _ob4[0][:])

        def issue_oproj(c, groups=range(16)):
            for g in groups:
                oproj_group(c, g // 4, g % 4, po, "psout")

        for c in range(NCH):
            if c == 0:
                xt = xt_c0_phaseC[0]
            else:
                xt = xp.tile([P, KO, CS], BF16, tag="xt", name="xt")
                nc.sync.dma_start(xt[:, :, :CS // 2], xT_r[:, :, c * CS:c * CS + CS // 2])
                nc.sync.dma_start(xt[:, :, CS // 2:], xT_r[:, :, c * CS + CS // 2:(c + 1) * CS])
            ctx_bf = stream.tile([P, NHL, CS], BF16, tag="ctx", name="ctx_bf")
            _ctx_of[c] = ctx_bf
            for h in range(NHL):
                psp = pa.tile([P, CS], F32, tag="pa", name="psp")
                if c == 0 and h == 0:
                    # split over s-halves: starts as soon as the first halves of
                    # Wphi and x(c0) have landed
                    for sh in range(2):
                        ssl = slice(sh * (CS // 2), (sh + 1) * (CS // 2))
                        for ko in range(KO):
                            nc.tensor.matmul(
                                psp[:, ssl], Wphi_sb[:, ko, h * HD:(h + 1) * HD],
                                xt[:, ko, ssl], start=(ko == 0), stop=(ko == KO - 1))
                else:
                    for ko in range(KO):
                        nc.tensor.matmul(
                            psp[:], Wphi_sb[:, ko, h * HD:(h + 1) * HD], xt[:, ko, :],
                            start=(ko == 0), stop=(ko == KO - 1))
                if c == 0:
                    # pending ropes, logits, softmax ride inside chunk 0's phi groups
                    if h == 0:
                        flush_all_pending()
                    elif h == 1:
                        issue_logits()
                    elif h == 2:
                        issue_softmax1()
                        issue_softmax2()
                        prime_kka(3)
                else:
                    # previous chunk's o_proj rides between this chunk's phi
                    # head-groups so its psum->sbuf copies never stall PE
                    issue_oproj(c - 1, range(h * 4, h * 4 + 4))
                phiT = stream.tile([P, CS], BF16, tag="phiT", name="phiT", bufs=4)
                nc.scalar.activation(phiT[:], psp[:], mybir.ActivationFunctionType.Identity,
                                     bias=bphi_sb[:, h:h + 1])
                if c > 0:
                    psr = pr.tile([P, CS], F32, tag="psr", name="psr")
                    nc.tensor.matmul(psr[:], outer_bf[:, h, :],
                                     QkT[:, h, c * CS:(c + 1) * CS], start=True, stop=True)
                    nc.vector.tensor_mul(ctx_bf[:, h, :], phiT[:], psr[:])
                else:
                    _phiT_c0 = _ctx_of.setdefault("phiT_c0", [])
                    _phiT_c0.append(phiT)
            if c == 0:
                issue_outer_all()
                for h in range(NHL):
                    psr = pr.tile([P, CS], F32, tag="psr", name="psr")
                    nc.tensor.matmul(psr[:], outer_bf[:, h, :],
                                     QkT[:, h, :CS], start=True, stop=True)
                    nc.vector.tensor_mul(ctx_bf[:, h, :], _ctx_of["phiT_c0"][h][:], psr[:])
        # final chunk's o_proj: alternate psum between po and the now-idle pa pool
        # for deeper buffering (the copy latency never blocks the matmuls)
        for g in range(16):
            oproj_group(NCH - 1, g // 4, g % 4, (po, pa)[g % 2], ("psout", "pa")[g % 2])

    nc.compile()
    return nc


def _host_prep(hidden_states, position_ids, Wq, Wk, Wv, Wo, Wphi, bphi):
    B = hidden_states.shape[0]
    # rope tables (match reference fp32 math)
    inv_freq = (1.0 / (ROPE_THETA ** (np.arange(0, HD, 2, dtype=np.float32) / HD))).astype(np.float32)
    in_maps = []
    Rm = np.zeros((P, P), dtype=np.float32)
    Rm[np.arange(64), np.arange(64) + 64] = -1.0
    Rm[np.arange(64) + 64, np.arange(64)] = 1.0
    RT_np = np.ascontiguousarray(Rm.T).astype(BF)
    for b in range(B):
        freqs = position_ids[b].astype(np.float32)[:, None] * inv_freq[None, :]
        emb = np.concatenate([freqs, freqs], axis=1)          # [S, 128]
        cos_b = np.cos(emb).astype(np.float32)
        sin_b = np.sin(emb).astype(np.float32)
        xT_b = np.ascontiguousarray(hidden_states[b].T).astype(BF)
        cosT_b = np.ascontiguousarray(cos_b.T)
        sinT_b = np.ascontiguousarray(sin_b.T)
        for g in range(4):
            sl4 = slice(g * 512, (g + 1) * 512)
            sl1 = slice(g * 128, (g + 1) * 128)
            in_maps.append({
                "xT": xT_b,
                "csT2": np.ascontiguousarray(np.stack([cosT_b, sinT_b], axis=1)),
                "css_sd": np.ascontiguousarray(
                    np.concatenate([cos_b[:, :64], sin_b[:, :64]], axis=1)),
                "Wq": np.ascontiguousarray(Wq[:, sl4]).astype(BF),
                "Wkv": np.ascontiguousarray(
                    np.concatenate([Wk[:, sl1], Wv[:, sl1]], axis=1)).astype(BF),
                "Wphi": np.ascontiguousarray(Wphi[:, sl4]).astype(BF),
                "Wo": np.ascontiguousarray(Wo[sl4, :]).astype(BF),
                "bphi": np.ascontiguousarray(bphi[sl4]).astype(np.float32),
                "RT": RT_np,
            })
    return in_maps


def kernel(hidden_states, position_ids, Wq, Wk, Wv, Wo, Wphi, bphi, _trace=False):
    if "nc" not in _CACHE:
        _CACHE["nc"] = _build()
    nc = _CACHE["nc"]
    in_maps = _host_prep(np.asarray(hidden_states), np.asarray(position_ids),
                         np.asarray(Wq), np.asarray(Wk), np.asarray(Wv),
                         np.asarray(Wo), np.asarray(Wphi), np.asarray(bphi))
    res = run_bass_kernel_spmd(nc, in_maps, list(range(8)), trace=_trace)
    _CACHE["last_res"] = res
    B = hidden_states.shape[0]
    out = np.empty((B, S, HID), dtype=np.float32)
    for b in range(B):
        acc = res.results[b * 4 + 0]["out"].astype(np.float32)
        for g in range(1, 4):
            acc = acc + res.results[b * 4 + g]["out"].astype(np.float32)
        out[b] = acc
    return out
